# revision 24
# baseline (speedup 1.0000x reference)
"""Trainium2 Bass kernel for nn_BlockDiagonalLinear_text (hyperbolic block-diag linear).

Math: the reference's per-row operations reduce to
  out = alpha_row * y   with  y = x @ blockdiag(W_1..W_16).T
where alpha_row is a chain of tanh/artanh scalars of ||x_row|| and
||y_row|| (the expmap0 scale cancels; validated numerically at 1.6e-4).

Sharding: data-parallel over rows; 8192 rows -> 8 cores x 1024 rows,
weights replicated (bf16).

Per-core pipeline (8 tiles of 128 rows), all-bf16 datapath:
  SWDGE cast-DMA x (fp32 HBM -> bf16 SBUF) ->
  PE transpose x chunks (bf16, 1 cyc/row) -> DVE copy xt (bf16 2x mode) ->
  PE: per-chunk Gram matmul accumulates x@x^T (diag = ||x||^2, extracted
  with one DVE tensor_tensor_reduce against an identity mask) +
  block matmuls y = x @ W^T (bf16, fp32 PSUM) ->
  ACT copies y PSUM->SBUF (cast bf16) -> DVE tensor_tensor_reduce y*y
  for ||y||^2 -> per-row scalar chain batched over tile PAIRS ([128,2]
  ops; Ln/Exp only, single ACT table set preloaded once) ->
  DVE in-place scale (bf16 4x mode) -> SWDGE cast-DMA out (bf16 -> fp32).
"""
import sys
import numpy as np

for _p in ("/opt/trn_rl_repo", "/root/.axon_site/_ro/trn_rl_repo"):
    if _p not in sys.path:
        sys.path.append(_p)

import ml_dtypes
import concourse.bass as bass
import concourse.bacc as bacc
import concourse.mybir as mybir
from concourse import tile
from concourse.bass_utils import run_bass_kernel_spmd
from concourse.hw_specs import get_activation_tables

R, BS = 16, 256           # 16 diagonal blocks of 256x256
D = R * BS                # 4096
P = 128                   # partitions
N_CORES = 8
ROWS_TOTAL = 4 * 2048     # 8192
ROWS_CORE = ROWS_TOTAL // N_CORES   # 1024
NT = ROWS_CORE // P       # 8 tiles of 128 rows per core
NC = D // P               # 32 k-chunks of 128
WCOLS = 2 * R * BS        # 8192 weight columns
WIDC = WCOLS + P          # + bf16 identity columns

f32 = mybir.dt.float32
bf16 = mybir.dt.bfloat16
AF = mybir.ActivationFunctionType
OP = mybir.AluOpType

CLIP_Z = float(np.float32(1.0) - np.float32(1e-5))          # 0.99999
MAXNORM = float(np.float32(1.0 - 1e-3) / np.float32(0.1))   # 9.99
# artanh(min(tanh(t), c)) == min(t, artanh(c)) -- the clamps collapse to
# min-with-constant, removing both tanh+artanh evaluations from the chain
ATH_CLIPZ = float(np.arctanh(np.float64(CLIP_Z)))           # 6.1030
ATH_MAXN = float(np.arctanh(np.float64(np.float32(0.1) * np.float32(MAXNORM))))


def build_nc():
    nc = bacc.Bacc()
    x_d = nc.declare_dram_parameter("x", [ROWS_CORE, D], f32, isOutput=False)
    w_d = nc.declare_dram_parameter("w", [P, WCOLS], bf16, isOutput=False)
    i_d = nc.declare_dram_parameter("idb", [P, P], bf16, isOutput=False)
    m_d = nc.declare_dram_parameter("idm", [P, P], f32, isOutput=False)
    out_d = nc.declare_dram_parameter("out", [ROWS_CORE, D], f32, isOutput=True)

    tabs = list(get_activation_tables(nc.m.arch).items())
    nle_id = next(i for i, (n, _) in enumerate(tabs)
                  if n == "natural_log_exp_and_others")

    with tile.TileContext(nc) as tc:
        with (
            tc.tile_pool(name="wpool", bufs=1) as wpool,
            tc.tile_pool(name="xpool", bufs=NT // 2) as xpool,
            tc.tile_pool(name="xtpool", bufs=2) as xtpool,
            tc.tile_pool(name="ypool", bufs=3) as ypool,
            tc.tile_pool(name="sqpool", bufs=2) as sqpool,
            tc.tile_pool(name="stats", bufs=3) as stats,
            tc.tile_pool(name="pst", bufs=2, space="PSUM") as pst,
            tc.tile_pool(name="psy", bufs=2, space="PSUM") as psy,
            tc.tile_pool(name="psg", bufs=1, space="PSUM") as psg,
        ):
            V = nc.vector

            # ACT: preload the one table set with ln+exp+copy so the
            # auto-inserted per-function loads (which thrash between the
            # natural_log and exp_and_others sets) all become no-ops.
            nc.scalar.add_instruction(mybir.InstLoadActFuncSet(
                name=nc.get_next_instruction_name(),
                act_func_set_id=nle_id, ins=[], outs=[]))

            # small identity first so tile-0 transposes start ~1us in,
            # while the 2MB weight DMA is still streaming
            id_sb = wpool.tile([P, P], bf16, name="id_sb")
            nc.sync.dma_start(out=id_sb[:], in_=i_d[:])
            idm_sb = wpool.tile([P, P], f32, name="idm_sb")
            nc.sync.dma_start(out=idm_sb[:], in_=m_d[:])
            w_sb = wpool.tile([P, WCOLS], bf16, name="w_sb")
            nc.sync.dma_start(out=w_sb[:], in_=w_d[:])

            def st(shape, tag):
                return stats.tile(shape, f32, tag=tag, name=tag)

            # Front-load the x-in cast-DMAs at PAIR granularity (fewer,
            # larger SWDGE ops -> less gpsimd queue serialization). Row
            # layout per pair buffer: partition p, slot s holds DRAM row
            # pair*256 + s*128 + p; the out-DMA mirrors it so the row
            # permutation cancels. First pair is split into two
            # tile-sized DMAs so compute starts sooner.
            xps = []
            for pr in range(NT // 2):
                xp = xpool.tile([P, 2 * D], bf16, tag="x", name=f"x_{pr}")
                src = x_d[pr * 2 * P:(pr + 1) * 2 * P, :].rearrange(
                    "(s p) d -> p s d", p=P)
                if pr == 0:
                    nc.gpsimd.dma_start(out=xp[:, 0:D], in_=src[:, 0, :])
                    nc.gpsimd.dma_start(out=xp[:, D:2 * D], in_=src[:, 1, :])
                else:
                    nc.gpsimd.dma_start(out=xp[:], in_=src)
                xps.append(xp)

            qq = None
            for i in range(NT):
                t = i % 2
                xb = xps[i // 2][:, t * D:(t + 1) * D]
                if t == 0:
                    qq = st([P, 4], "qq")   # [qx_t0, qx_t1, qy_t0, qy_t1]

                # transpose x: 4 chunks of 128 per PSUM tile, then one
                # bf16 2x-mode DVE copy per group of 4
                xt = xtpool.tile([P, D], bf16, tag="xt", name=f"xt_{i}")
                gram = psg.tile([P, P], f32, tag="gram", name=f"gram_{i}")
                if t == 0:
                    y_pair = ypool.tile([P, 2 * D], bf16, tag="y",
                                        name=f"y_{i // 2}")
                y_sb = y_pair[:, t * D:(t + 1) * D]
                for g in range(NC // 4):
                    tp = pst.tile([P, 4 * P], bf16, tag="tp", name=f"tp_{i}_{g}")
                    for c in range(4):
                        kc = 4 * g + c
                        nc.tensor.transpose(
                            tp[:, c * P:(c + 1) * P],
                            xb[:, kc * P:(kc + 1) * P], id_sb)
                    V.tensor_copy(xt[:, g * 4 * P:(g + 1) * 4 * P], tp[:])
                    # Gram: accumulate x @ x^T over all 32 chunks; its
                    # diagonal is the row-wise ||x||^2
                    for c in range(4):
                        kc = 4 * g + c
                        nc.tensor.matmul(
                            gram[:],
                            xt[:, kc * P:(kc + 1) * P],
                            xt[:, kc * P:(kc + 1) * P],
                            start=(kc == 0), stop=(kc == NC - 1),
                        )
                    # y block matmuls for the 2 blocks covered by this group
                    if g % 2 == 1:
                        py = psy.tile([P, 4 * BS], f32, tag="py",
                                      name=f"py_{i}_{g // 2}")
                        for rr in range(4):
                            r = 4 * (g // 2) + rr
                            for c in range(2):
                                kc = 2 * r + c
                                nc.tensor.matmul(
                                    py[:, rr * BS:(rr + 1) * BS],
                                    xt[:, kc * P:(kc + 1) * P],
                                    w_sb[:, kc * BS:(kc + 1) * BS],
                                    start=(c == 0), stop=(c == 1),
                                )
                        # drain 4 blocks at once: ACT copy PSUM -> SBUF bf16
                        nc.scalar.activation(
                            y_sb[:, (g // 2) * 4 * BS:(g // 2 + 1) * 4 * BS],
                            py[:], AF.Copy)

                # qx = diag(gram): mask with identity, then free-dim reduce
                gsc = sqpool.tile([P, P], f32, tag="gsc", name=f"gsc_{i}")
                V.tensor_mul(gsc[:], gram[:], idm_sb[:])
                V.reduce_sum(qq[:, t:t + 1], gsc[:], axis=mybir.AxisListType.X)
                # qy = sum y^2 on ACT, in halves so the first half overlaps
                # the remaining PSUM drains (Square is in the preloaded set)
                sq = sqpool.tile([P, D], bf16, tag="sq", name=f"sq_{i}")
                qp = st([P, 2], f"qp_{t}")
                for h in range(2):
                    nc.scalar.activation(sq[:, h * (D // 2):(h + 1) * (D // 2)],
                                         y_sb[:, h * (D // 2):(h + 1) * (D // 2)],
                                         AF.Square, accum_out=qp[:, h:h + 1])
                V.tensor_add(qq[:, 2 + t:3 + t], qp[:, 0:1], qp[:, 1:2])

                if t == 0:
                    continue

                # ---- per-row scalar chain for the tile pair ([128,2]) ----
                lnq = st([P, 4], "lnq")
                nc.scalar.activation(lnq[:], qq[:], AF.Ln)
                U = st([P, 4], "U")     # [u | y_n] = sqrt via exp(0.5 ln q)
                nc.scalar.activation(U[:], lnq[:], AF.Exp, scale=0.5)

                t1 = st([P, 2], "t1")   # 0.1 * max(u, 1e-5)
                V.tensor_scalar(out=t1[:], in0=U[:, 0:2], scalar1=1e-5,
                                scalar2=0.1, op0=OP.max, op1=OP.mult)
                r1 = st([P, 2], "r1")
                V.reciprocal(r1[:], t1[:])
                d_ = st([P, 2], "d_")     # 2*artanh(min(tanh(t1), CLIP_Z))
                V.tensor_scalar(out=d_[:], in0=t1[:], scalar1=ATH_CLIPZ,
                                scalar2=2.0, op0=OP.min, op1=OP.mult)
                yns = st([P, 2], "yns")
                V.tensor_scalar_max(yns[:], U[:, 2:4], 1e-20)
                w1 = st([P, 2], "w1")
                V.tensor_mul(w1[:], U[:, 2:4], r1[:])
                w2 = st([P, 2], "w2")
                V.tensor_mul(w2[:], w1[:], d_[:])
                argt = st([P, 2], "argt")
                V.tensor_scalar(out=argt[:], in0=w2[:], scalar1=0.05,
                                scalar2=15.0, op0=OP.mult, op1=OP.min)
                Et = st([P, 2], "Et")
                nc.scalar.activation(Et[:], argt[:], AF.Exp, scale=2.0)
                e2 = st([P, 2], "e2")
                V.tensor_scalar_add(e2[:], Et[:], 1.0)
                r3 = st([P, 2], "r3")
                V.reciprocal(r3[:], e2[:])
                ttx = st([P, 2], "ttx")   # tanh(arg_t)
                V.tensor_scalar(out=ttx[:], in0=r3[:], scalar1=-2.0,
                                scalar2=1.0, op0=OP.mult, op1=OP.add)
                nrm = st([P, 2], "nrm")
                V.tensor_scalar(out=nrm[:], in0=ttx[:], scalar1=10.0,
                                scalar2=1e-5, op0=OP.mult, op1=OP.max)
                # project+logmap collapse: pf/zb == 10/nrm in both branches,
                # so alpha = ttx/yns * 2*artanh(0.1*min(nrm,9.99)) * 10/nrm
                # with the artanh again collapsing to min(argt, ATH_MAXN)
                ryn = st([P, 2], "ryn")
                V.reciprocal(ryn[:], yns[:])
                rn = st([P, 2], "rn")
                V.reciprocal(rn[:], nrm[:])
                db = st([P, 2], "db")
                V.tensor_scalar(out=db[:], in0=argt[:], scalar1=ATH_MAXN,
                                scalar2=2.0, op0=OP.min, op1=OP.mult)
                a1 = st([P, 2], "a1")
                V.tensor_mul(a1[:], ttx[:], ryn[:])
                a2 = st([P, 2], "a2")
                V.tensor_mul(a2[:], db[:], rn[:])
                al = st([P, 2], "al")
                V.tensor_mul(al[:], a1[:], a2[:])
                mask = st([P, 2], "mask")
                V.tensor_scalar(out=mask[:], in0=qq[:, 2:4], scalar1=0.0,
                                scalar2=None, op0=OP.is_gt)
                alm = st([P, 2], "alm")
                V.tensor_mul(alm[:], al[:], mask[:])

                # scale both tiles in place (bf16 4x mode), then cast-DMA
                # out per tile (bf16 -> fp32); the factor 50 folds the
                # logmap 10/nrm and the artanh halves
                for tt in (0, 1):
                    yt = y_pair[:, tt * D:(tt + 1) * D]
                    V.tensor_scalar(out=yt, in0=yt,
                                    scalar1=alm[:, tt:tt + 1], scalar2=50.0,
                                    op0=OP.mult, op1=OP.mult)
                    ii = i - 1 + tt
                    nc.gpsimd.dma_start(out=out_d[ii * P:(ii + 1) * P, :],
                                        in_=yt)
    nc.finalize()
    return nc


_NC = None


def _get_nc():
    global _NC
    if _NC is None:
        _NC = build_nc()
    return _NC


def _prep_weights(weights: np.ndarray) -> np.ndarray:
    # w_sb[p, (2r+c)*256+j] = W[r, j, k=c*128+p]; bf16.
    wt = (weights.astype(np.float32).transpose(0, 2, 1)      # [r, k, j]
          .reshape(R, 2, P, BS).transpose(2, 0, 1, 3)        # [p, r, c, j]
          .reshape(P, WCOLS))
    return np.ascontiguousarray(wt).astype(ml_dtypes.bfloat16)


def _in_maps(x: np.ndarray, weights: np.ndarray) -> list:
    xf = np.ascontiguousarray(x, dtype=np.float32).reshape(ROWS_TOTAL, D)
    wid = _prep_weights(np.asarray(weights))
    idb = np.eye(P, dtype=ml_dtypes.bfloat16)
    idm = np.eye(P, dtype=np.float32)
    return [
        {"x": xf[i * ROWS_CORE:(i + 1) * ROWS_CORE], "w": wid,
         "idb": idb, "idm": idm}
        for i in range(N_CORES)
    ]


def kernel(x: np.ndarray, weights: np.ndarray) -> np.ndarray:
    nc = _get_nc()
    in_maps = _in_maps(x, weights)
    res = run_bass_kernel_spmd(nc, in_maps, list(range(N_CORES)))
    out = np.concatenate([res.results[i]["out"] for i in range(N_CORES)], axis=0)
    return out.reshape(x.shape).astype(np.float32, copy=False)


if __name__ == "__main__":
    xs = np.random.randn(4, 2048, D).astype(np.float32)
    ws = (np.broadcast_to(np.eye(BS, dtype=np.float32), (R, BS, BS))
          + 0.02 * np.random.randn(R, BS, BS).astype(np.float32))
    o = kernel(xs, ws)
    print("kernel ran, out shape", o.shape, o.dtype)


# revision 25
# speedup vs baseline: 1.2415x; 1.2415x over previous
"""Trainium2 Bass kernel for nn_BlockDiagonalLinear_text (hyperbolic block-diag linear).

Math: the reference's per-row operations reduce to
  out = alpha_row * y   with  y = x @ blockdiag(W_1..W_16).T
where alpha_row is a chain of tanh/artanh scalars of ||x_row|| and
||y_row|| (the expmap0 scale cancels; validated numerically at 1.6e-4).

Sharding: data-parallel over rows; 8192 rows -> 8 cores x 1024 rows,
weights replicated (bf16).

Per-core pipeline (8 tiles of 128 rows), all-bf16 datapath:
  SWDGE cast-DMA x (fp32 HBM -> bf16 SBUF) ->
  PE transpose x chunks (bf16, 1 cyc/row) -> DVE copy xt (bf16 2x mode) ->
  PE: per-chunk Gram matmul accumulates x@x^T (diag = ||x||^2, extracted
  with one DVE tensor_tensor_reduce against an identity mask) +
  block matmuls y = x @ W^T (bf16, fp32 PSUM) ->
  ACT copies y PSUM->SBUF (cast bf16) -> DVE tensor_tensor_reduce y*y
  for ||y||^2 -> per-row scalar chain batched over tile PAIRS ([128,2]
  ops; Ln/Exp only, single ACT table set preloaded once) ->
  DVE in-place scale (bf16 4x mode) -> SWDGE cast-DMA out (bf16 -> fp32).
"""
import sys
import numpy as np

for _p in ("/opt/trn_rl_repo", "/root/.axon_site/_ro/trn_rl_repo"):
    if _p not in sys.path:
        sys.path.append(_p)

import ml_dtypes
import concourse.bass as bass
import concourse.bacc as bacc
import concourse.mybir as mybir
from concourse import tile
from concourse.bass_utils import run_bass_kernel_spmd
from concourse.hw_specs import get_activation_tables

R, BS = 16, 256           # 16 diagonal blocks of 256x256
D = R * BS                # 4096
P = 128                   # partitions
N_CORES = 8
ROWS_TOTAL = 4 * 2048     # 8192
ROWS_CORE = ROWS_TOTAL // N_CORES   # 1024
NT = ROWS_CORE // P       # 8 tiles of 128 rows per core
NC = D // P               # 32 k-chunks of 128
WCOLS = 2 * R * BS        # 8192 weight columns
WIDC = WCOLS + P          # + bf16 identity columns

f32 = mybir.dt.float32
bf16 = mybir.dt.bfloat16
AF = mybir.ActivationFunctionType
OP = mybir.AluOpType

CLIP_Z = float(np.float32(1.0) - np.float32(1e-5))          # 0.99999
MAXNORM = float(np.float32(1.0 - 1e-3) / np.float32(0.1))   # 9.99
# artanh(min(tanh(t), c)) == min(t, artanh(c)) -- the clamps collapse to
# min-with-constant, removing both tanh+artanh evaluations from the chain
ATH_CLIPZ = float(np.arctanh(np.float64(CLIP_Z)))           # 6.1030
ATH_MAXN = float(np.arctanh(np.float64(np.float32(0.1) * np.float32(MAXNORM))))


def build_nc():
    nc = bacc.Bacc()
    x_d = nc.declare_dram_parameter("x", [ROWS_CORE, D], f32, isOutput=False)
    w_d = nc.declare_dram_parameter("w", [P, WCOLS], bf16, isOutput=False)
    i_d = nc.declare_dram_parameter("idb", [P, P], bf16, isOutput=False)
    m_d = nc.declare_dram_parameter("idm", [P, P], f32, isOutput=False)
    out_d = nc.declare_dram_parameter("out", [ROWS_CORE, D], f32, isOutput=True)

    tabs = list(get_activation_tables(nc.m.arch).items())
    nle_id = next(i for i, (n, _) in enumerate(tabs)
                  if n == "natural_log_exp_and_others")

    with tile.TileContext(nc) as tc:
        with (
            tc.tile_pool(name="wpool", bufs=1) as wpool,
            tc.tile_pool(name="xpool", bufs=NT // 2) as xpool,
            tc.tile_pool(name="xtpool", bufs=2) as xtpool,
            tc.tile_pool(name="ypool", bufs=3) as ypool,
            tc.tile_pool(name="sqpool", bufs=2) as sqpool,
            tc.tile_pool(name="stats", bufs=3) as stats,
            tc.tile_pool(name="pst", bufs=3, space="PSUM") as pst,
            tc.tile_pool(name="psy", bufs=2, space="PSUM") as psy,
            tc.tile_pool(name="psg", bufs=1, space="PSUM") as psg,
        ):
            V = nc.vector

            # ACT: preload the one table set with ln+exp+copy so the
            # auto-inserted per-function loads (which thrash between the
            # natural_log and exp_and_others sets) all become no-ops.
            nc.scalar.add_instruction(mybir.InstLoadActFuncSet(
                name=nc.get_next_instruction_name(),
                act_func_set_id=nle_id, ins=[], outs=[]))

            # small identity first so tile-0 transposes start ~1us in,
            # while the 2MB weight DMA is still streaming
            id_sb = wpool.tile([P, P], bf16, name="id_sb")
            nc.sync.dma_start(out=id_sb[:], in_=i_d[:])
            idm_sb = wpool.tile([P, P], f32, name="idm_sb")
            nc.sync.dma_start(out=idm_sb[:], in_=m_d[:])
            w_sb = wpool.tile([P, WCOLS], bf16, name="w_sb")
            nc.sync.dma_start(out=w_sb[:], in_=w_d[:])

            def st(shape, tag):
                return stats.tile(shape, f32, tag=tag, name=tag)

            # Front-load the x-in cast-DMAs at PAIR granularity (fewer,
            # larger SWDGE ops -> less gpsimd queue serialization). Row
            # layout per pair buffer: partition p, slot s holds DRAM row
            # pair*256 + s*128 + p; the out-DMA mirrors it so the row
            # permutation cancels. First pair is split into two
            # tile-sized DMAs so compute starts sooner.
            xps = []
            for pr in range(NT // 2):
                xp = xpool.tile([P, 2 * D], bf16, tag="x", name=f"x_{pr}")
                src = x_d[pr * 2 * P:(pr + 1) * 2 * P, :].rearrange(
                    "(s p) d -> p s d", p=P)
                if pr == 0:
                    nc.gpsimd.dma_start(out=xp[:, 0:D], in_=src[:, 0, :])
                    nc.gpsimd.dma_start(out=xp[:, D:2 * D], in_=src[:, 1, :])
                else:
                    nc.gpsimd.dma_start(out=xp[:], in_=src)
                xps.append(xp)

            def emit_chain(qq, c, scale_outs):
                # qq: [P, 2c] = [qx cols | qy cols]; scale_outs: list of
                # (y_slice, out_row_base) per column
                lnq = st([P, 2 * c], "lnq")
                nc.scalar.activation(lnq[:], qq[:], AF.Ln)
                U = st([P, 2 * c], "U")   # [u | y_n] = sqrt via exp(.5 ln q)
                nc.scalar.activation(U[:], lnq[:], AF.Exp, scale=0.5)
                t1 = st([P, c], "t1")     # 0.1 * max(u, 1e-5)
                V.tensor_scalar(out=t1[:], in0=U[:, 0:c], scalar1=1e-5,
                                scalar2=0.1, op0=OP.max, op1=OP.mult)
                r1 = st([P, c], "r1")
                V.reciprocal(r1[:], t1[:])
                d_ = st([P, c], "d_")     # 2*artanh(min(tanh(t1), CLIP_Z))
                V.tensor_scalar(out=d_[:], in0=t1[:], scalar1=ATH_CLIPZ,
                                scalar2=2.0, op0=OP.min, op1=OP.mult)
                yns = st([P, c], "yns")
                V.tensor_scalar_max(yns[:], U[:, c:2 * c], 1e-20)
                w1 = st([P, c], "w1")
                V.tensor_mul(w1[:], U[:, c:2 * c], r1[:])
                w2 = st([P, c], "w2")
                V.tensor_mul(w2[:], w1[:], d_[:])
                argt = st([P, c], "argt")
                V.tensor_scalar(out=argt[:], in0=w2[:], scalar1=0.05,
                                scalar2=15.0, op0=OP.mult, op1=OP.min)
                Et = st([P, c], "Et")
                nc.scalar.activation(Et[:], argt[:], AF.Exp, scale=2.0)
                e2 = st([P, c], "e2")
                V.tensor_scalar_add(e2[:], Et[:], 1.0)
                r3 = st([P, c], "r3")
                V.reciprocal(r3[:], e2[:])
                ttx = st([P, c], "ttx")   # tanh(arg_t)
                V.tensor_scalar(out=ttx[:], in0=r3[:], scalar1=-2.0,
                                scalar2=1.0, op0=OP.mult, op1=OP.add)
                nrm = st([P, c], "nrm")
                V.tensor_scalar(out=nrm[:], in0=ttx[:], scalar1=10.0,
                                scalar2=1e-5, op0=OP.mult, op1=OP.max)
                # project+logmap collapse: pf/zb == 10/nrm in both branches;
                # the artanh again collapses to min(argt, ATH_MAXN)
                ryn = st([P, c], "ryn")
                V.reciprocal(ryn[:], yns[:])
                rn = st([P, c], "rn")
                V.reciprocal(rn[:], nrm[:])
                db = st([P, c], "db")
                V.tensor_scalar(out=db[:], in0=argt[:], scalar1=ATH_MAXN,
                                scalar2=2.0, op0=OP.min, op1=OP.mult)
                a1 = st([P, c], "a1")
                V.tensor_mul(a1[:], ttx[:], ryn[:])
                a2 = st([P, c], "a2")
                V.tensor_mul(a2[:], db[:], rn[:])
                al = st([P, c], "al")
                V.tensor_mul(al[:], a1[:], a2[:])
                mask = st([P, c], "mask")
                V.tensor_scalar(out=mask[:], in0=qq[:, c:2 * c], scalar1=0.0,
                                scalar2=None, op0=OP.is_gt)
                alm = st([P, c], "alm")
                V.tensor_mul(alm[:], al[:], mask[:])
                # scale in place (bf16 4x mode) + per-tile cast-DMA out;
                # factor 50 folds the logmap 10/nrm and the artanh halves
                for cc, (yt, row) in enumerate(scale_outs):
                    V.tensor_scalar(out=yt, in0=yt,
                                    scalar1=alm[:, cc:cc + 1], scalar2=50.0,
                                    op0=OP.mult, op1=OP.mult)
                    nc.gpsimd.dma_start(out=out_d[row:row + P, :], in_=yt)

            qq = None
            for i in range(NT):
                t = i % 2
                last_pair = (i // 2 == NT // 2 - 1)
                xb = xps[i // 2][:, t * D:(t + 1) * D]
                if last_pair:
                    qq = st([P, 2], f"qqs{t}")   # per-tile [qx, qy]
                elif t == 0:
                    qq = st([P, 4], "qq")   # [qx_t0, qx_t1, qy_t0, qy_t1]

                # transpose x: 4 chunks of 128 per PSUM tile, then one
                # bf16 2x-mode DVE copy per group of 4
                xt = xtpool.tile([P, D], bf16, tag="xt", name=f"xt_{i}")
                gram = psg.tile([P, P], f32, tag="gram", name=f"gram_{i}")
                if t == 0:
                    y_pair = ypool.tile([P, 2 * D], bf16, tag="y",
                                        name=f"y_{i // 2}")
                y_sb = y_pair[:, t * D:(t + 1) * D]
                for g in range(NC // 4):
                    tp = pst.tile([P, 4 * P], bf16, tag="tp", name=f"tp_{i}_{g}")
                    for c in range(4):
                        kc = 4 * g + c
                        nc.tensor.transpose(
                            tp[:, c * P:(c + 1) * P],
                            xb[:, kc * P:(kc + 1) * P], id_sb)
                    V.tensor_copy(xt[:, g * 4 * P:(g + 1) * 4 * P], tp[:])
                    # Gram: accumulate x @ x^T over all 32 chunks; its
                    # diagonal is the row-wise ||x||^2
                    for c in range(4):
                        kc = 4 * g + c
                        nc.tensor.matmul(
                            gram[:],
                            xt[:, kc * P:(kc + 1) * P],
                            xt[:, kc * P:(kc + 1) * P],
                            start=(kc == 0), stop=(kc == NC - 1),
                        )
                    # y block matmuls for the 2 blocks covered by this group
                    if g % 2 == 1:
                        py = psy.tile([P, 4 * BS], f32, tag="py",
                                      name=f"py_{i}_{g // 2}")
                        for rr in range(4):
                            r = 4 * (g // 2) + rr
                            for c in range(2):
                                kc = 2 * r + c
                                nc.tensor.matmul(
                                    py[:, rr * BS:(rr + 1) * BS],
                                    xt[:, kc * P:(kc + 1) * P],
                                    w_sb[:, kc * BS:(kc + 1) * BS],
                                    start=(c == 0), stop=(c == 1),
                                )
                        # drain 4 blocks at once: ACT copy PSUM -> SBUF bf16
                        nc.scalar.activation(
                            y_sb[:, (g // 2) * 4 * BS:(g // 2 + 1) * 4 * BS],
                            py[:], AF.Copy)

                # qx = diag(gram): mask with identity, then free-dim reduce
                qxc = 0 if last_pair else t
                nq = 1 if last_pair else 2
                gsc = sqpool.tile([P, P], f32, tag="gsc", name=f"gsc_{i}")
                V.tensor_mul(gsc[:], gram[:], idm_sb[:])
                V.reduce_sum(qq[:, qxc:qxc + 1], gsc[:],
                             axis=mybir.AxisListType.X)
                # qy = sum y^2 on ACT, in halves so the first half overlaps
                # the remaining PSUM drains (Square is in the preloaded set)
                sq = sqpool.tile([P, D], bf16, tag="sq", name=f"sq_{i}")
                qp = st([P, 2], f"qp_{t}")
                for h in range(2):
                    nc.scalar.activation(sq[:, h * (D // 2):(h + 1) * (D // 2)],
                                         y_sb[:, h * (D // 2):(h + 1) * (D // 2)],
                                         AF.Square, accum_out=qp[:, h:h + 1])
                V.tensor_add(qq[:, nq + qxc:nq + qxc + 1],
                             qp[:, 0:1], qp[:, 1:2])

                if last_pair:
                    # per-tile chain so this tile's output streams without
                    # waiting for its pair partner (shrinks the DMA tail)
                    emit_chain(qq, 1, [(y_sb, i * P)])
                    continue
                if t == 0:
                    continue

                # ---- per-row scalar chain for the tile pair ([128,2]) ----
                emit_chain(qq, 2, [
                    (y_pair[:, 0:D], (i - 1) * P),
                    (y_pair[:, D:2 * D], i * P),
                ])
    nc.finalize()
    return nc


_NC = None


def _get_nc():
    global _NC
    if _NC is None:
        _NC = build_nc()
    return _NC


def _prep_weights(weights: np.ndarray) -> np.ndarray:
    # w_sb[p, (2r+c)*256+j] = W[r, j, k=c*128+p]; bf16.
    wt = (weights.astype(np.float32).transpose(0, 2, 1)      # [r, k, j]
          .reshape(R, 2, P, BS).transpose(2, 0, 1, 3)        # [p, r, c, j]
          .reshape(P, WCOLS))
    return np.ascontiguousarray(wt).astype(ml_dtypes.bfloat16)


def _in_maps(x: np.ndarray, weights: np.ndarray) -> list:
    xf = np.ascontiguousarray(x, dtype=np.float32).reshape(ROWS_TOTAL, D)
    wid = _prep_weights(np.asarray(weights))
    idb = np.eye(P, dtype=ml_dtypes.bfloat16)
    idm = np.eye(P, dtype=np.float32)
    return [
        {"x": xf[i * ROWS_CORE:(i + 1) * ROWS_CORE], "w": wid,
         "idb": idb, "idm": idm}
        for i in range(N_CORES)
    ]


def kernel(x: np.ndarray, weights: np.ndarray) -> np.ndarray:
    nc = _get_nc()
    in_maps = _in_maps(x, weights)
    res = run_bass_kernel_spmd(nc, in_maps, list(range(N_CORES)))
    out = np.concatenate([res.results[i]["out"] for i in range(N_CORES)], axis=0)
    return out.reshape(x.shape).astype(np.float32, copy=False)


if __name__ == "__main__":
    xs = np.random.randn(4, 2048, D).astype(np.float32)
    ws = (np.broadcast_to(np.eye(BS, dtype=np.float32), (R, BS, BS))
          + 0.02 * np.random.randn(R, BS, BS).astype(np.float32))
    o = kernel(xs, ws)
    print("kernel ran, out shape", o.shape, o.dtype)
